# revision 11
# baseline (speedup 1.0000x reference)
"""Causal multi-head attention (B=2, H=16, S=2048, D=128) on 8 TRN2 NeuronCores.

Sharding: batch*heads (32) split across 8 cores, 4 heads per core.
Per-head algorithm (fp16 matmuls / f32 accumulation), v5:
  - chunked HWDGE f32 loads for Q,K (4 tiles per chunk, separate SBUF tiles so
    dependency tracking is per-chunk); V via SWDGE cast-DMA (f32 -> fp16) with
    a ones column appended
  - DVE pre-cast f32 -> fp16 per chunk, then PE-transpose fp16 tiles (1 cy/row)
    to [d, s] layout; PSUM -> SBUF copy on DVE
  - scores computed transposed: S^T[k, q] so the PV matmul needs no P transpose
  - full k-chunks in 3-tile PSUM groups [128,1536]; the 4 diagonal k-chunks of
    each 512-q block as 4 row-matmuls packed tight [128,1280] (valid cols only)
  - P^T = exp(S^T/sqrt(D)) on ScalarE straight from PSUM -> SBUF fp16; no
    max-subtraction needed (scores ~N(0,1); the reference's -10000 mask
    underflows to exact 0 in exp, so hard zeros match it)
  - causal diagonal tiles masked by a 0/1 triangle multiply (GpSimd) after exp
  - O = sum_k P^T.T @ V_aug with a ones column appended to V -> last column of
    the accumulator is the softmax denominator; DVE reciprocal + tensor_scalar
  - per-q-block f32 stores; O-phase of each block emitted one scores-unit late
    so PE keeps ScalarE fed across block/head boundaries; head-0 loads staged
    chunk-by-chunk so the first scores start ASAP
"""

import math

import numpy as np

import concourse.bass as bass
import concourse.tile as tile
from concourse import bacc, mybir
from concourse.bass_utils import run_bass_kernel_spmd

B, H, S, D = 2, 16, 2048, 128
N_CORES = 8
HPC = (B * H) // N_CORES  # heads per core
P = 128                   # partitions / head_dim / k-chunk
NT = S // P               # 16 k-chunks (s-tiles) per head
QB = 512                  # q-block width
NQB = S // QB             # 4 q-blocks per head
G = 3                     # full-group tiles per exp (3 PSUM banks)

FP16 = mybir.dt.float16
F32 = mybir.dt.float32
EXPFN = mybir.ActivationFunctionType.Exp
SCALE = 1.0 / math.sqrt(D)

_cache = {}


def _build_program():
    """Build (once) the single-core Bass/Tile program used SPMD on all cores."""
    if "nc" in _cache:
        return _cache["nc"]

    nc = bacc.Bacc("TRN2", target_bir_lowering=False, debug=False)

    q_d = nc.dram_tensor("q", [HPC * S, D], F32, kind="ExternalInput").ap()
    k_d = nc.dram_tensor("k", [HPC * S, D], F32, kind="ExternalInput").ap()
    v_d = nc.dram_tensor("v", [HPC * S, D], F32, kind="ExternalInput").ap()
    ident_d = nc.dram_tensor("ident", [P, P], FP16, kind="ExternalInput").ap()
    tri_d = nc.dram_tensor("tri", [P, P], FP16, kind="ExternalInput").ap()
    o_d = nc.dram_tensor("o", [HPC * S, D], F32, kind="ExternalOutput").ap()

    with tile.TileContext(nc) as tc:
        with (
            tc.tile_pool(name="consts", bufs=1) as consts,
            tc.tile_pool(name="qn", bufs=6) as qn_pool,
            tc.tile_pool(name="kn", bufs=6) as kn_pool,
            tc.tile_pool(name="qh", bufs=4) as qh_pool,
            tc.tile_pool(name="kh", bufs=4) as kh_pool,
            tc.tile_pool(name="qt", bufs=8) as qt_pool,
            tc.tile_pool(name="kt", bufs=8) as kt_pool,
            tc.tile_pool(name="vt", bufs=8) as vt_pool,
            tc.tile_pool(name="ptf", bufs=2) as ptf_pool,
            tc.tile_pool(name="ptd", bufs=2) as ptd_pool,
            tc.tile_pool(name="ostage", bufs=4) as ostage_pool,
            tc.tile_pool(name="rec", bufs=4) as rec_pool,
            tc.tile_pool(name="stp", bufs=2, space="PSUM") as st_pool,
            tc.tile_pool(name="ops", bufs=2, space="PSUM") as o_pool,
        ):
            ident = consts.tile([P, P], FP16)
            nc.sync.dma_start(ident[:], ident_d[:])
            tri = consts.tile([P, P], FP16)
            nc.sync.dma_start(tri[:], tri_d[:])

            heads = [dict() for _ in range(HPC)]

            def emit_load_chunk(h, c):
                t = heads[h]
                if c == 0:
                    t["qn"], t["kn"], t["vt"] = [], [], []
                rows = slice(h * S, (h + 1) * S)
                cs = slice(4 * c, 4 * c + 4)
                k_h = k_d[rows, :].rearrange("(n p) d -> p n d", p=P)
                q_h = q_d[rows, :].rearrange("(n p) d -> p n d", p=P)
                v_h = v_d[rows, :].rearrange("(n p) d -> p n d", p=P)
                knc = kn_pool.tile([P, 4, P], F32, name=f"kn{h}_{c}", tag="kn")
                nc.sync.dma_start(knc[:], k_h[:, cs, :])
                qnc = qn_pool.tile([P, 4, P], F32, name=f"qn{h}_{c}", tag="qn")
                nc.sync.dma_start(qnc[:], q_h[:, cs, :])
                vtc = vt_pool.tile([P, 4, P + 1], FP16, name=f"vt{h}_{c}", tag="vt")
                nc.gpsimd.dma_start(vtc[:, :, 0:P], v_h[:, cs, :])  # f32->fp16
                nc.vector.memset(vtc[:, :, P : P + 1], 1.0)
                t["kn"].append(knc)
                t["qn"].append(qnc)
                t["vt"].append(vtc)

            def emit_trans_chunk(h, c):
                t = heads[h]
                if c == 0:
                    t["qt"], t["kt"] = [], []
                # DVE pre-cast f32 -> fp16 so the PE transpose runs 1 cy/row
                qhc = qh_pool.tile([P, 4, P], FP16, name=f"qh{h}_{c}", tag="qh")
                nc.vector.tensor_copy(qhc[:], t["qn"][c][:])
                khc = kh_pool.tile([P, 4, P], FP16, name=f"kh{h}_{c}", tag="kh")
                nc.vector.tensor_copy(khc[:], t["kn"][c][:])
                # [128, 2, 1024] fp16 = 2 psum banks: q block in bank 0,
                # k block in bank 1 (copies don't stall transposes)
                ts = st_pool.tile([P, 2, 2 * QB], FP16, name=f"ts{h}_{c}", tag="stp")
                for i in range(4):
                    nc.tensor.transpose(
                        ts[:, 0, i * P : (i + 1) * P], qhc[:, i, :], ident[:]
                    )
                for i in range(4):
                    nc.tensor.transpose(
                        ts[:, 1, i * P : (i + 1) * P], khc[:, i, :], ident[:]
                    )
                qtc = qt_pool.tile([P, 4, P], FP16, name=f"qt{h}_{c}", tag="qt")
                ktc = kt_pool.tile([P, 4, P], FP16, name=f"kt{h}_{c}", tag="kt")
                nc.vector.tensor_copy(qtc[:], ts[:, 0, 0:QB])
                nc.vector.tensor_copy(ktc[:], ts[:, 1, 0:QB])
                t["qt"].append(qtc)
                t["kt"].append(ktc)

            def emit_scores(h, b):
                t = heads[h]
                qt, kt = t["qt"], t["kt"]
                qt_b = qt[b]  # [128, 4, 128] = [128, 512] moving operand
                nfull = 4 * b
                # --- full (entirely valid) k-chunks in G-tile psum groups ---
                ptf = None
                if nfull:
                    ptf = ptf_pool.tile(
                        [P, 12, QB], FP16, name=f"ptf{h}_{b}", tag="ptf"
                    )
                    for gs in range(0, nfull, G):
                        gw = min(G, nfull - gs)
                        stp = st_pool.tile(
                            [P, G, QB], F32, name=f"st{h}_{b}_{gs}", tag="stp"
                        )
                        for jj in range(gw):
                            j = gs + jj
                            nc.tensor.matmul(
                                stp[:, jj, :],
                                lhsT=kt[j // 4][:, j % 4, :],
                                rhs=qt_b[:],
                                start=True,
                                stop=True,
                            )
                        nc.scalar.activation(
                            ptf[:, gs : gs + gw, :],
                            stp[:, 0:gw, :],
                            EXPFN,
                            scale=SCALE,
                        )
                # --- diagonal k-chunks: one row-matmul per r, packed tight ---
                # row order (0,3,1,2) packs to exactly 1280 cols with no
                # matmul crossing a 512-f32 psum bank boundary
                dstp = st_pool.tile([P, 1280], F32, name=f"dst{h}_{b}", tag="stp")
                ptd = ptd_pool.tile([P, 1280], FP16, name=f"ptd{h}_{b}", tag="ptd")
                dcol = {}
                col = 0
                for r in (0, 3, 1, 2):
                    w = QB - P * r
                    nc.tensor.matmul(
                        dstp[:, col : col + w],
                        lhsT=kt[b][:, r, :],
                        rhs=qt_b[:, r:4, :],
                        start=True,
                        stop=True,
                    )
                    dcol[r] = col
                    col += w
                nc.scalar.activation(ptd[:], dstp[:], EXPFN, scale=SCALE)
                for r in range(4):
                    dslc = ptd[:, dcol[r] : dcol[r] + P]
                    nc.gpsimd.tensor_mul(dslc, dslc, tri[:])
                t[("pt", b)] = (ptf, ptd, dcol)

            def emit_out(h, b):
                t = heads[h]
                vt = t["vt"]
                ptf, ptd, dcol = t.pop(("pt", b))
                ostage = ostage_pool.tile(
                    [P, 4, P], F32, name=f"os{h}_{b}", tag="ostage"
                )
                nfull = 4 * b
                otiles = [
                    o_pool.tile([P, 2, P + 1], F32, name=f"o{h}_{b}_{i}", tag="ops")
                    for i in range(2)
                ]
                for s in range(4):
                    ot = otiles[s // 2][:, s % 2, :]
                    for j in range(nfull):
                        nc.tensor.matmul(
                            ot,
                            lhsT=ptf[:, j, s * P : (s + 1) * P],
                            rhs=vt[j // 4][:, j % 4, :],
                            start=(j == 0),
                            stop=False,
                        )
                    for r in range(s + 1):
                        c0 = dcol[r] + (s - r) * P
                        nc.tensor.matmul(
                            ot,
                            lhsT=ptd[:, c0 : c0 + P],
                            rhs=vt[b][:, r, :],
                            start=(nfull == 0 and r == 0),
                            stop=(r == s),
                        )
                rows = slice(h * S, (h + 1) * S)
                o_h = o_d[rows, :].rearrange("(n p) d -> p n d", p=P)
                for s in range(4):
                    ot = otiles[s // 2][:, s % 2, :]
                    rec = rec_pool.tile([P, 1], F32, name=f"r{h}_{b}_{s}", tag="rec")
                    nc.vector.reciprocal(rec[:], ot[:, P : P + 1])
                    nc.vector.tensor_scalar_mul(ostage[:, s, :], ot[:, 0:P], rec[:])
                bs = slice(4 * b, 4 * b + 4)
                nc.sync.dma_start(o_h[:, bs, :], ostage[:])

            # head 0 is staged chunk-by-chunk so scores start on chunk 0
            emit_load_chunk(0, 0)
            emit_trans_chunk(0, 0)
            pending = None
            # head h+1: loads at block 0, transpose chunks spread over blocks
            tsched = {1: [0], 2: [1, 2], 3: [3]}
            for h in range(HPC):
                for b in range(NQB):
                    if h == 0 and b > 0:
                        emit_load_chunk(0, b)
                        emit_trans_chunk(0, b)
                    if h + 1 < HPC:
                        for c in tsched.get(b, []):
                            emit_trans_chunk(h + 1, c)
                    emit_scores(h, b)
                    if pending is not None:
                        emit_out(*pending)
                    pending = (h, b)
                    if b == 0 and h + 1 < HPC:
                        for c in range(4):
                            emit_load_chunk(h + 1, c)
            emit_out(*pending)

    nc.compile()
    _cache["nc"] = nc
    return nc


def _make_const_inputs():
    ident = np.eye(P, dtype=np.float16)
    # tri[kk, qq] = 1 where qq >= kk (valid causal positions in S^T layout)
    tri = np.triu(np.ones((P, P), dtype=np.float16))
    return ident, tri


def run_sharded(q, k, v, trace=False, **kw):
    """q,k,v: [B,H,S,D] f32 -> (out [B,H,S,D] f32, BassKernelResults)."""
    nc = _build_program()
    qf = np.ascontiguousarray(np.asarray(q, dtype=np.float32).reshape(B * H, S, D))
    kf = np.ascontiguousarray(np.asarray(k, dtype=np.float32).reshape(B * H, S, D))
    vf = np.ascontiguousarray(np.asarray(v, dtype=np.float32).reshape(B * H, S, D))
    ident, tri = _make_const_inputs()
    in_maps = []
    for c in range(N_CORES):
        hs = slice(c * HPC, (c + 1) * HPC)
        in_maps.append(
            {
                "q": qf[hs].reshape(HPC * S, D),
                "k": kf[hs].reshape(HPC * S, D),
                "v": vf[hs].reshape(HPC * S, D),
                "ident": ident,
                "tri": tri,
            }
        )
    res = run_bass_kernel_spmd(nc, in_maps, list(range(N_CORES)), trace=trace, **kw)
    outs = [res.results[c]["o"].reshape(HPC, S, D) for c in range(N_CORES)]
    full = np.concatenate(outs, axis=0).reshape(B, H, S, D)
    return full, res


def kernel(query_states, key_states, value_states):
    out, _ = run_sharded(query_states, key_states, value_states)
    return out.astype(np.float32)


# revision 14
# speedup vs baseline: 1.0040x; 1.0040x over previous
"""Causal multi-head attention (B=2, H=16, S=2048, D=128) on 8 TRN2 NeuronCores.

Sharding: batch*heads (32) split across 8 cores, 4 heads per core.
Per-head algorithm (fp16 matmuls / f32 accumulation), v5:
  - chunked HWDGE f32 loads for Q,K (4 tiles per chunk, separate SBUF tiles so
    dependency tracking is per-chunk); V via SWDGE cast-DMA (f32 -> fp16) with
    a ones column appended
  - DVE pre-cast f32 -> fp16 per chunk, then PE-transpose fp16 tiles (1 cy/row)
    to [d, s] layout; PSUM -> SBUF copy on DVE
  - scores computed transposed: S^T[k, q] so the PV matmul needs no P transpose
  - full k-chunks in 3-tile PSUM groups [128,1536]; the 4 diagonal k-chunks of
    each 512-q block as 4 row-matmuls packed tight [128,1280] (valid cols only)
  - P^T = exp(S^T/sqrt(D)) on ScalarE straight from PSUM -> SBUF fp16; no
    max-subtraction needed (scores ~N(0,1); the reference's -10000 mask
    underflows to exact 0 in exp, so hard zeros match it)
  - causal diagonal tiles masked by a 0/1 triangle multiply (GpSimd) after exp
  - O = sum_k P^T.T @ V_aug with a ones column appended to V -> last column of
    the accumulator is the softmax denominator; DVE reciprocal + tensor_scalar
  - per-q-block f32 stores; O-phase of each block emitted one scores-unit late
    so PE keeps ScalarE fed across block/head boundaries; head-0 loads staged
    chunk-by-chunk so the first scores start ASAP
"""

import math

import numpy as np

import concourse.bass as bass
import concourse.tile as tile
from concourse import bacc, mybir
from concourse.bass_utils import run_bass_kernel_spmd

B, H, S, D = 2, 16, 2048, 128
N_CORES = 8
HPC = (B * H) // N_CORES  # heads per core
P = 128                   # partitions / head_dim / k-chunk
NT = S // P               # 16 k-chunks (s-tiles) per head
QB = 512                  # q-block width
NQB = S // QB             # 4 q-blocks per head
G = 3                     # full-group tiles per exp (3 PSUM banks)

FP16 = mybir.dt.float16
F32 = mybir.dt.float32
EXPFN = mybir.ActivationFunctionType.Exp
SCALE = 1.0 / math.sqrt(D)

_cache = {}


def _build_program():
    """Build (once) the single-core Bass/Tile program used SPMD on all cores."""
    if "nc" in _cache:
        return _cache["nc"]

    nc = bacc.Bacc("TRN2", target_bir_lowering=False, debug=False)

    q_d = nc.dram_tensor("q", [HPC * S, D], F32, kind="ExternalInput").ap()
    k_d = nc.dram_tensor("k", [HPC * S, D], F32, kind="ExternalInput").ap()
    v_d = nc.dram_tensor("v", [HPC * S, D], F32, kind="ExternalInput").ap()
    ident_d = nc.dram_tensor("ident", [P, P], FP16, kind="ExternalInput").ap()
    tri_d = nc.dram_tensor("tri", [P, P], FP16, kind="ExternalInput").ap()
    o_d = nc.dram_tensor("o", [HPC * S, D], F32, kind="ExternalOutput").ap()

    with tile.TileContext(nc) as tc:
        with (
            tc.tile_pool(name="consts", bufs=1) as consts,
            tc.tile_pool(name="qn", bufs=6) as qn_pool,
            tc.tile_pool(name="kn", bufs=6) as kn_pool,
            tc.tile_pool(name="qh", bufs=4) as qh_pool,
            tc.tile_pool(name="kh", bufs=4) as kh_pool,
            tc.tile_pool(name="qt", bufs=8) as qt_pool,
            tc.tile_pool(name="kt", bufs=8) as kt_pool,
            tc.tile_pool(name="vt", bufs=8) as vt_pool,
            tc.tile_pool(name="ptf", bufs=2) as ptf_pool,
            tc.tile_pool(name="ptd", bufs=2) as ptd_pool,
            tc.tile_pool(name="ostage", bufs=4) as ostage_pool,
            tc.tile_pool(name="rec", bufs=4) as rec_pool,
            tc.tile_pool(name="stp", bufs=2, space="PSUM") as st_pool,
            tc.tile_pool(name="ops", bufs=2, space="PSUM") as o_pool,
        ):
            ident = consts.tile([P, P], FP16)
            nc.sync.dma_start(ident[:], ident_d[:])
            tri = consts.tile([P, P], FP16)
            nc.sync.dma_start(tri[:], tri_d[:])

            heads = [dict() for _ in range(HPC)]

            def emit_load_chunk(h, c, with_v=True):
                t = heads[h]
                if c == 0:
                    t["qn"], t["kn"] = [], []
                rows = slice(h * S, (h + 1) * S)
                cs = slice(4 * c, 4 * c + 4)
                k_h = k_d[rows, :].rearrange("(n p) d -> p n d", p=P)
                q_h = q_d[rows, :].rearrange("(n p) d -> p n d", p=P)
                knc = kn_pool.tile([P, 4, P], F32, name=f"kn{h}_{c}", tag="kn")
                nc.sync.dma_start(knc[:], k_h[:, cs, :])
                qnc = qn_pool.tile([P, 4, P], F32, name=f"qn{h}_{c}", tag="qn")
                nc.sync.dma_start(qnc[:], q_h[:, cs, :])
                t["kn"].append(knc)
                t["qn"].append(qnc)
                if with_v:
                    emit_v_chunk(h, c)

            def emit_v_chunk(h, c):
                t = heads[h]
                if c == 0:
                    t["vt"] = []
                rows = slice(h * S, (h + 1) * S)
                cs = slice(4 * c, 4 * c + 4)
                v_h = v_d[rows, :].rearrange("(n p) d -> p n d", p=P)
                vtc = vt_pool.tile([P, 4, P + 1], FP16, name=f"vt{h}_{c}", tag="vt")
                nc.gpsimd.dma_start(vtc[:, :, 0:P], v_h[:, cs, :])  # f32->fp16
                nc.vector.memset(vtc[:, :, P : P + 1], 1.0)
                t["vt"].append(vtc)

            def emit_trans_chunk(h, c):
                t = heads[h]
                if c == 0:
                    t["qt"], t["kt"] = [], []
                # DVE pre-cast f32 -> fp16 so the PE transpose runs 1 cy/row
                qhc = qh_pool.tile([P, 4, P], FP16, name=f"qh{h}_{c}", tag="qh")
                nc.vector.tensor_copy(qhc[:], t["qn"][c][:])
                khc = kh_pool.tile([P, 4, P], FP16, name=f"kh{h}_{c}", tag="kh")
                nc.vector.tensor_copy(khc[:], t["kn"][c][:])
                # [128, 2, 1024] fp16 = 2 psum banks: q block in bank 0,
                # k block in bank 1 (copies don't stall transposes)
                ts = st_pool.tile([P, 2, 2 * QB], FP16, name=f"ts{h}_{c}", tag="stp")
                for i in range(4):
                    nc.tensor.transpose(
                        ts[:, 0, i * P : (i + 1) * P], qhc[:, i, :], ident[:]
                    )
                for i in range(4):
                    nc.tensor.transpose(
                        ts[:, 1, i * P : (i + 1) * P], khc[:, i, :], ident[:]
                    )
                qtc = qt_pool.tile([P, 4, P], FP16, name=f"qt{h}_{c}", tag="qt")
                ktc = kt_pool.tile([P, 4, P], FP16, name=f"kt{h}_{c}", tag="kt")
                nc.vector.tensor_copy(qtc[:], ts[:, 0, 0:QB])
                nc.vector.tensor_copy(ktc[:], ts[:, 1, 0:QB])
                t["qt"].append(qtc)
                t["kt"].append(ktc)

            def emit_scores(h, b):
                t = heads[h]
                qt, kt = t["qt"], t["kt"]
                qt_b = qt[b]  # [128, 4, 128] = [128, 512] moving operand
                nfull = 4 * b
                # --- full (entirely valid) k-chunks in G-tile psum groups ---
                ptf = None
                if nfull:
                    ptf = ptf_pool.tile(
                        [P, 12, QB], FP16, name=f"ptf{h}_{b}", tag="ptf"
                    )
                    for gs in range(0, nfull, G):
                        gw = min(G, nfull - gs)
                        stp = st_pool.tile(
                            [P, G, QB], F32, name=f"st{h}_{b}_{gs}", tag="stp"
                        )
                        for jj in range(gw):
                            j = gs + jj
                            nc.tensor.matmul(
                                stp[:, jj, :],
                                lhsT=kt[j // 4][:, j % 4, :],
                                rhs=qt_b[:],
                                start=True,
                                stop=True,
                            )
                        nc.scalar.activation(
                            ptf[:, gs : gs + gw, :],
                            stp[:, 0:gw, :],
                            EXPFN,
                            scale=SCALE,
                        )
                # --- diagonal k-chunks: one row-matmul per r, packed tight ---
                # row order (0,3,1,2) packs to exactly 1280 cols with no
                # matmul crossing a 512-f32 psum bank boundary
                dstp = st_pool.tile([P, 1280], F32, name=f"dst{h}_{b}", tag="stp")
                ptd = ptd_pool.tile([P, 1280], FP16, name=f"ptd{h}_{b}", tag="ptd")
                dcol = {}
                col = 0
                for r in (0, 3, 1, 2):
                    w = QB - P * r
                    nc.tensor.matmul(
                        dstp[:, col : col + w],
                        lhsT=kt[b][:, r, :],
                        rhs=qt_b[:, r:4, :],
                        start=True,
                        stop=True,
                    )
                    dcol[r] = col
                    col += w
                nc.scalar.activation(ptd[:], dstp[:], EXPFN, scale=SCALE)
                for r in range(4):
                    dslc = ptd[:, dcol[r] : dcol[r] + P]
                    nc.gpsimd.tensor_mul(dslc, dslc, tri[:])
                t[("pt", b)] = (ptf, ptd, dcol)

            def emit_out(h, b):
                t = heads[h]
                vt = t["vt"]
                ptf, ptd, dcol = t.pop(("pt", b))
                ostage = ostage_pool.tile(
                    [P, 4, P], F32, name=f"os{h}_{b}", tag="ostage"
                )
                nfull = 4 * b
                otiles = [
                    o_pool.tile([P, 2, P + 1], F32, name=f"o{h}_{b}_{i}", tag="ops")
                    for i in range(2)
                ]
                for s in range(4):
                    ot = otiles[s // 2][:, s % 2, :]
                    for j in range(nfull):
                        nc.tensor.matmul(
                            ot,
                            lhsT=ptf[:, j, s * P : (s + 1) * P],
                            rhs=vt[j // 4][:, j % 4, :],
                            start=(j == 0),
                            stop=False,
                        )
                    for r in range(s + 1):
                        c0 = dcol[r] + (s - r) * P
                        nc.tensor.matmul(
                            ot,
                            lhsT=ptd[:, c0 : c0 + P],
                            rhs=vt[b][:, r, :],
                            start=(nfull == 0 and r == 0),
                            stop=(r == s),
                        )
                rows = slice(h * S, (h + 1) * S)
                o_h = o_d[rows, :].rearrange("(n p) d -> p n d", p=P)
                for s in range(4):
                    ot = otiles[s // 2][:, s % 2, :]
                    rec = rec_pool.tile([P, 1], F32, name=f"r{h}_{b}_{s}", tag="rec")
                    nc.vector.reciprocal(rec[:], ot[:, P : P + 1])
                    nc.vector.tensor_scalar_mul(ostage[:, s, :], ot[:, 0:P], rec[:])
                bs = slice(4 * b, 4 * b + 4)
                nc.sync.dma_start(o_h[:, bs, :], ostage[:])

            # head 0 is staged chunk-by-chunk so scores start on chunk 0
            emit_load_chunk(0, 0)
            emit_trans_chunk(0, 0)
            pending = None
            # head h+1: loads at block 0, transpose chunks spread over blocks
            tsched = {1: [0], 2: [1, 2], 3: [3]}
            for h in range(HPC):
                for b in range(NQB):
                    if h == 0 and b > 0:
                        emit_load_chunk(0, b)
                        emit_trans_chunk(0, b)
                    if h + 1 < HPC:
                        for c in tsched.get(b, []):
                            emit_trans_chunk(h + 1, c)
                    emit_scores(h, b)
                    if pending is not None:
                        emit_out(*pending)
                    pending = (h, b)
                    if b == 0 and h + 1 < HPC:
                        for c in range(4):
                            emit_load_chunk(h + 1, c)
            emit_out(*pending)

    nc.compile()
    _cache["nc"] = nc
    return nc


def _make_const_inputs():
    ident = np.eye(P, dtype=np.float16)
    # tri[kk, qq] = 1 where qq >= kk (valid causal positions in S^T layout)
    tri = np.triu(np.ones((P, P), dtype=np.float16))
    return ident, tri


def run_sharded(q, k, v, trace=False, **kw):
    """q,k,v: [B,H,S,D] f32 -> (out [B,H,S,D] f32, BassKernelResults)."""
    nc = _build_program()
    qf = np.ascontiguousarray(np.asarray(q, dtype=np.float32).reshape(B * H, S, D))
    kf = np.ascontiguousarray(np.asarray(k, dtype=np.float32).reshape(B * H, S, D))
    vf = np.ascontiguousarray(np.asarray(v, dtype=np.float32).reshape(B * H, S, D))
    ident, tri = _make_const_inputs()
    in_maps = []
    for c in range(N_CORES):
        hs = slice(c * HPC, (c + 1) * HPC)
        in_maps.append(
            {
                "q": qf[hs].reshape(HPC * S, D),
                "k": kf[hs].reshape(HPC * S, D),
                "v": vf[hs].reshape(HPC * S, D),
                "ident": ident,
                "tri": tri,
            }
        )
    res = run_bass_kernel_spmd(nc, in_maps, list(range(N_CORES)), trace=trace, **kw)
    outs = [res.results[c]["o"].reshape(HPC, S, D) for c in range(N_CORES)]
    full = np.concatenate(outs, axis=0).reshape(B, H, S, D)
    return full, res


def kernel(query_states, key_states, value_states):
    out, _ = run_sharded(query_states, key_states, value_states)
    return out.astype(np.float32)
